# revision 4
# baseline (speedup 1.0000x reference)
"""Boundary-map kernel for Trainium2 (Bass, raw engine streams), 8-core SPMD.

Math: a pixel is an edge pixel iff its radius-2 Euclidean disk (clipped to the
zero-padded label image) contains two different labels; with DH/DV the
horizontal/vertical not-equal maps, edge = (sum of 16 tap-shifted DH/DV
terms) > 0. Vertical taps are band-matrix matmuls on the PE; horizontal taps
are DVE shifted adds plus column-offset rhs slices.

Per-core layout: seg1/seg2 = two 128-partition row bands (124 output rows
each, 2-row halo), strip = bottom 32 rows of BOTH batches packed as
[72 x 260] (full-lane use). Host supplies next-row copies so DV is a plain
free-dim not_equal.

Input: labels ship as INT8 and are expanded to bf16 in flight by Pool-SWDGE
casting DMAs (fan out over all 16 DMA engines, halve HBM traffic; HWDGE
rings carry only the weights and strip tile). PE runs warm-up dummy matmuls
so its DVFS ramp (~3us to full clock) completes before real passes arrive.
DVE spine: both segs' H2 paths first, then the DV paths, feeding PE's
interleaved 5-pass groups (w11*H2(0), wi*H2(+-1), wv4*DV(0), wv2*DVHp(-1));
strip is 4-pass into pB bank 3.

Output: thresholds produce 0/1 bf16 (ACT Sign + DVE is_gt, split to finish
together); the PE then BIT-PACKS 8 rows per byte via power-of-2 weight
matmuls into retired PSUM banks (out base partitions limited to {0,32,64} ->
3 slots x 3 regions), one uint8 copy (ACT+DVE halves), and three tiny
16-partition output DMAs on the three queues. DRAM writes cap at ~63 GB/s
per core, so shrinking the output 8x is what kills the tail. The host
unpacks bits. 7 manual semaphores, no TileContext.

Measured: ~30.0-30.9 us vs the 36.5 us baseline (runs occasionally land in
a ~1.2x-downclocked DVFS state; compare structure via traces, not raw ns).
"""

import numpy as np
import ml_dtypes

import concourse.bass as bass
import concourse.bacc as bacc
import concourse.mybir as mybir
from concourse import bass_utils

BF16 = mybir.dt.bfloat16
F32 = mybir.dt.float32
I8 = mybir.dt.int8
U8 = mybir.dt.uint8
OP = mybir.AluOpType
AF = mybir.ActivationFunctionType

B, H, W = 2, 1024, 2048
NCORES = 8
CHUNK = 512
SEGW = 2052
STW = 260
T = 2 * SEGW           # strip base col in intermediates
IW = 2 * SEGW + STW

PROFILE = False
LAST_EXEC_NS = None
LAST_RESULTS = None

N_DUMMY = 22


def _band(taps, P=128):
    w = np.zeros((P, P), np.float32)
    for m in range(P):
        for t in taps:
            k = m + t
            if 0 <= k < P:
                w[k, m] += 1.0
    return w


def _sband(taps):
    w36 = _band(taps, P=36)
    w = np.zeros((72, 72), np.float32)
    w[0:36, 0:36] = w36
    w[36:72, 36:72] = w36
    return w


TAPS = {"w_11": [-1, 1], "w_v4": [-2, -1, 0, 1], "w_v2": [-1, 0], "w_i": [0]}
WNAMES = ("w_11", "w_v4", "w_v2", "w_i")


def make_weights():
    big = np.concatenate([_band(TAPS[n]) for n in WNAMES], axis=1)
    strip = np.zeros((128, 72 * len(WNAMES)), np.float32)
    for i, n in enumerate(WNAMES):
        strip[0:72, 72 * i:72 * (i + 1)] = _sband(TAPS[n])
    pack = np.zeros((128, 16), np.float32)
    for i in range(16):
        for j in range(8):
            pack[8 * i + j, i] = float(1 << j)
    return np.concatenate([big, strip, pack], axis=1).astype(ml_dtypes.bfloat16)


def build_nc():
    nc = bacc.Bacc("TRN2", target_bir_lowering=False, debug=False)
    x8 = nc.dram_tensor("x8", [128, 4 * SEGW], I8, kind="ExternalInput").ap()
    st = nc.dram_tensor("st", [72, 2 * STW], BF16, kind="ExternalInput").ap()
    wcat = nc.dram_tensor("wcat", [128, 512 + 72 * 4 + 16], BF16,
                          kind="ExternalInput").ap()
    y2p = nc.dram_tensor("y2p", [48, 1536], U8, kind="ExternalOutput").ap()

    xi = nc.alloc_sbuf_tensor("xi", [128, 4 * SEGW], BF16)
    ST = nc.alloc_sbuf_tensor("ST", [72, 2 * STW], BF16)
    wt = nc.alloc_sbuf_tensor("wt", [128, 512 + 72 * 4 + 16], BF16)
    DH = nc.alloc_sbuf_tensor("DH", [128, IW], BF16)
    H2 = nc.alloc_sbuf_tensor("H2", [128, IW], BF16)
    DV = nc.alloc_sbuf_tensor("DV", [128, IW], BF16)
    DVHp = nc.alloc_sbuf_tensor("DVHp", [128, IW], BF16)
    H4p = nc.alloc_sbuf_tensor("H4p", [128, IW], BF16)
    e1 = nc.alloc_sbuf_tensor("e1", [128, 4352], BF16)
    e1p = nc.alloc_sbuf_tensor("e1p", [128, 1536], U8)
    pA = nc.alloc_psum_tensor("pA", [128, 2048], F32)
    pB = nc.alloc_psum_tensor("pB", [128, 2048], F32)

    wb = {n: wt[:, 128 * i:128 * (i + 1)] for i, n in enumerate(WNAMES)}
    ws = {n: wt[0:72, 512 + 72 * i:512 + 72 * (i + 1)]
          for i, n in enumerate(WNAMES)}
    wpk = wt[:, 800:816]

    s1 = nc.alloc_semaphore("s1")        # SP ring (wcat)
    s2 = nc.alloc_semaphore("s2")        # ACT ring (strip)
    s3 = nc.alloc_semaphore("s3")        # SWDGE casting DMAs
    vsem = nc.alloc_semaphore("vsem")
    psem = nc.alloc_semaphore("psem")
    asem = nc.alloc_semaphore("asem")
    osem = nc.alloc_semaphore("osem")

    with nc.Block(no_gpsimd_drain=True) as blk:

        @blk.sync
        def _(eng):
            eng.dma_start(wt[:, :], wcat).then_inc(s1, 16)
            eng.wait_ge(asem, 7)
            eng.wait_ge(vsem, 18)
            eng.dma_start(y2p[16:32, :], e1p[32:48, :]).then_inc(osem, 16)
            eng.wait_ge(osem, 48)

        @blk.gpsimd
        def _(eng):
            # SWDGE casting DMAs: int8 HBM -> bf16 SBUF, fan out over all
            # 16 DMA engines; cur halves first (they gate the DH/H2 path)
            eng.dma_start(xi[:, 0:SEGW], x8[:, 0:SEGW]).then_inc(s3, 16)
            eng.dma_start(xi[:, 2 * SEGW:3 * SEGW],
                          x8[:, 2 * SEGW:3 * SEGW]).then_inc(s3, 16)
            eng.dma_start(xi[:, SEGW:2 * SEGW],
                          x8[:, SEGW:2 * SEGW]).then_inc(s3, 16)
            eng.dma_start(xi[:, 3 * SEGW:4 * SEGW],
                          x8[:, 3 * SEGW:4 * SEGW]).then_inc(s3, 16)
            # bit-packed output, partition-group slice on each queue
            eng.wait_ge(asem, 7)
            eng.wait_ge(vsem, 18)
            eng.dma_start(y2p[0:16, :], e1p[0:16, :]).then_inc(osem, 16)

        @blk.scalar
        def _(eng):
            eng.dma_start(ST[:, :], st[:, :]).then_inc(s2, 16)
            # threshold copies split with DVE: ACT takes strip (retires pB
            # bank 3 for seg2 c3) + c0/c1 of each seg; DVE takes c2/c3
            eng.wait_ge(psem, 1)
            eng.activation(out=e1[0:72, 4096:4352], in_=pB[0:72, 1536:1792],
                           func=AF.Sign).then_inc(asem, 1)
            for k in range(2):
                eng.wait_ge(psem, 2 + k)
                eng.activation(out=e1[:, k * CHUNK:(k + 1) * CHUNK],
                               in_=pA[:, k * CHUNK:(k + 1) * CHUNK],
                               func=AF.Sign).then_inc(asem, 1)
            for k in range(2):
                eng.wait_ge(psem, 6 + k)
                eng.activation(out=e1[:, 2048 + k * CHUNK:2048 + (k + 1) * CHUNK],
                               in_=pB[:, k * CHUNK:(k + 1) * CHUNK],
                               func=AF.Sign).then_inc(asem, 1)
            # packed result -> uint8 (garbage in the partition gaps; the
            # host reads only the three 16-partition slices)
            eng.wait_ge(psem, 10)
            eng.copy(out=e1p[:, 0:512], in_=pA[:, 0:512]).then_inc(asem, 1)
            eng.wait_ge(psem, 11)
            eng.copy(out=e1p[:, 512:1024],
                     in_=pA[:, 512:1024]).then_inc(asem, 1)
            eng.wait_ge(vsem, 18)
            eng.dma_start(y2p[32:48, :], e1p[64:80, :]).then_inc(osem, 16)

        @blk.vector
        def _(eng):
            # strip chain first: its tiny input lands well before the big
            # casted segments, so it hides entirely in the input window
            eng.wait_ge(s2, 16)
            eng.tensor_tensor(out=DH[0:72, T:T + 259], in0=ST[:, 0:259],
                              in1=ST[:, 1:260],
                              op=OP.not_equal).then_inc(vsem, 1)   # v1 SDH
            eng.tensor_tensor(out=H2[0:72, T + 1:T + 259],
                              in0=DH[0:72, T:T + 258],
                              in1=DH[0:72, T + 1:T + 259],
                              op=OP.add).then_inc(vsem, 1)         # v2 SH2
            eng.tensor_tensor(out=DV[0:72, T:T + 260], in0=ST[:, 0:260],
                              in1=ST[:, 260:520],
                              op=OP.not_equal).then_inc(vsem, 1)   # v3 SDV
            eng.tensor_tensor(out=DVHp[0:72, T:T + 258],
                              in0=DV[0:72, T:T + 258],
                              in1=DV[0:72, T + 2:T + 260],
                              op=OP.add).then_inc(vsem, 1)         # v4 SDVHp
            eng.tensor_tensor(out=H4p[0:72, T + 1:T + 257],
                              in0=H2[0:72, T + 1:T + 257],
                              in1=H2[0:72, T + 3:T + 259],
                              op=OP.add).then_inc(vsem, 1)         # v5 SH4p
            # H2 paths for BOTH segs first (cur halves arrive first), then
            # the DV paths -- feeds PE's interleaved group order
            eng.wait_ge(s3, 16)
            eng.tensor_tensor(out=DH[:, 0:2051], in0=xi[:, 0:2051],
                              in1=xi[:, 1:2052],
                              op=OP.not_equal).then_inc(vsem, 1)   # v6 DH1
            eng.tensor_tensor(out=H2[:, 1:2051],
                              in0=DH[:, 0:2050],
                              in1=DH[:, 1:2051],
                              op=OP.add).then_inc(vsem, 1)         # v7 H2-1
            S = SEGW
            eng.wait_ge(s3, 32)
            eng.tensor_tensor(out=DH[:, S:S + 2051],
                              in0=xi[:, 2 * SEGW:2 * SEGW + 2051],
                              in1=xi[:, 2 * SEGW + 1:2 * SEGW + 2052],
                              op=OP.not_equal).then_inc(vsem, 1)   # v8 DH2
            eng.tensor_tensor(out=H2[:, S + 1:S + 2051],
                              in0=DH[:, S:S + 2050],
                              in1=DH[:, S + 1:S + 2051],
                              op=OP.add).then_inc(vsem, 1)         # v9 H2-2
            eng.wait_ge(s3, 48)
            eng.tensor_tensor(out=DV[:, 0:2052], in0=xi[:, 0:2052],
                              in1=xi[:, SEGW:SEGW + 2052],
                              op=OP.not_equal).then_inc(vsem, 1)   # v10 DV1
            eng.tensor_tensor(out=DVHp[:, 0:2050],
                              in0=DV[:, 0:2050],
                              in1=DV[:, 2:2052],
                              op=OP.add).then_inc(vsem, 1)         # v11 DVHp1
            eng.wait_ge(s3, 64)
            eng.tensor_tensor(out=DV[:, S:S + 2052],
                              in0=xi[:, 2 * SEGW:2 * SEGW + 2052],
                              in1=xi[:, 3 * SEGW:3 * SEGW + 2052],
                              op=OP.not_equal).then_inc(vsem, 1)   # v12 DV2
            eng.tensor_tensor(out=DVHp[:, S:S + 2050],
                              in0=DV[:, S:S + 2050],
                              in1=DV[:, S + 2:S + 2052],
                              op=OP.add).then_inc(vsem, 1)         # v13 DVHp2
            # c2/c3 threshold copies of both segs on the now-idle DVE, in
            # parallel with ACT's strip/c0/c1 copies
            for S, ps, pw in ((1024, pA, 4), (1536, pA, 5),
                              (3072, pB, 8), (3584, pB, 9)):
                eng.wait_ge(psem, pw)
                eng.tensor_scalar(out=e1[:, S:S + CHUNK],
                                  in0=ps[:, S % 2048:S % 2048 + CHUNK],
                                  scalar1=0.0, scalar2=None,
                                  op0=OP.is_gt).then_inc(vsem, 1)  # v14-17
            eng.wait_ge(psem, 12)
            eng.tensor_scalar(out=e1p[:, 1024:1536], in0=pA[:, 1024:1536],
                              scalar1=0.0, scalar2=None,
                              op0=OP.bypass).then_inc(vsem, 1)     # v18

        @blk.tensor
        def _(eng):
            for i in range(N_DUMMY):
                eng.matmul(out=pB[:, 1536:2048], lhsT=H4p[0:128, 512:640],
                           rhs=H4p[0:128, 1024:1536], start=True, stop=True,
                           skip_group_check=True)
            # strip: 4 passes into pB bank 3, right after the dummies; ACT
            # copies it out (asem 1) long before seg2-c3 reuses the bank
            eng.wait_ge(s1, 16)
            eng.wait_ge(vsem, 5)
            a = T + 2
            for wi, (wn, rhs, doff) in enumerate(
                    [("w_11", H2, 0), ("w_v4", DV, 0),
                     ("w_v2", DVHp, -1), ("w_i", H4p, -1)]):
                mm = eng.matmul(out=pB[0:72, 1536:1792], lhsT=ws[wn],
                                rhs=rhs[0:72, a + doff:a + doff + 256],
                                start=(wi == 0), stop=(wi == 3),
                                skip_group_check=True)
            mm.then_inc(psem, 1)

            def seg_group(seg, wn, rhs, doff, stop=False, p0=None):
                S, ps = (0, pA) if seg == 1 else (SEGW, pB)
                for k in range(4):
                    if seg == 2 and wi_guard[0] and k == 3:
                        eng.wait_ge(asem, 1)
                        wi_guard[0] = False
                    a = S + 2 + doff + k * CHUNK
                    mm = eng.matmul(out=ps[:, k * CHUNK:(k + 1) * CHUNK],
                                    lhsT=wb[wn],
                                    rhs=rhs[0:128, a:a + CHUNK],
                                    start=(p0 is not None and p0 == "start"),
                                    stop=stop, skip_group_check=True)
                    if stop:
                        mm.then_inc(psem, 1)

            wi_guard = [True]
            # interleaved: H2-dependent groups of both segs first, DV/DVHp
            # groups as DVE produces them
            eng.wait_ge(vsem, 7)
            seg_group(1, "w_11", H2, 0, p0="start")
            seg_group(1, "w_i", H2, -1)
            seg_group(1, "w_i", H2, 1)
            eng.wait_ge(vsem, 9)
            seg_group(2, "w_11", H2, 0, p0="start")
            seg_group(2, "w_i", H2, -1)
            seg_group(2, "w_i", H2, 1)
            eng.wait_ge(vsem, 10)
            seg_group(1, "w_v4", DV, 0)
            eng.wait_ge(vsem, 11)
            seg_group(1, "w_v2", DVHp, -1, stop=True)
            eng.wait_ge(vsem, 12)
            seg_group(2, "w_v4", DV, 0)
            eng.wait_ge(vsem, 13)
            seg_group(2, "w_v2", DVHp, -1, stop=True)
            # bit-pack passes: out rows 8i..8i+7 of each 512-col chunk fold
            # into byte-partition 16k+i of pA banks 0/1 (free after their
            # threshold reads)
            # matmul out base partitions limited to {0, 32, 64}: chunk ->
            # (slot, region) of pA banks 0-2, strip at (2, 2); banks are
            # reused only after their threshold copy retired them
            packs = [(0, 0, 0, "a", 2), (1, 1, 0, "a", 3), (2, 2, 0, "v", 14),
                     (8, 2, 2, None, 0), (3, 0, 1, "v", 15),
                     (4, 1, 1, "a", 4), (5, 2, 1, "a", 5),
                     (6, 0, 2, "v", 16), (7, 1, 2, "v", 17)]
            for pi, (k, slot, reg, sem, lvl) in enumerate(packs):
                if sem == "a":
                    eng.wait_ge(asem, lvl)
                elif sem == "v":
                    eng.wait_ge(vsem, lvl)
                base, off = 32 * slot, 512 * reg
                if k == 8:  # strip
                    mm = eng.matmul(out=pA[base:base + 9, off:off + 256],
                                    lhsT=wpk[0:72, 0:9],
                                    rhs=e1[0:72, 4096:4352], start=True,
                                    stop=True, skip_group_check=True)
                else:
                    mm = eng.matmul(out=pA[base:base + 16, off:off + 512],
                                    lhsT=wpk[0:128, 0:16],
                                    rhs=e1[0:128, 512 * k:512 * (k + 1)],
                                    start=True, stop=True,
                                    skip_group_check=True)
                if pi in (2, 6, 8):  # region 0 / 1 / 2 complete
                    mm.then_inc(psem, 1)

    nc.compile()
    return nc


def make_in_maps(gtmasks):
    lab8 = np.asarray(gtmasks)[:, 0].astype(np.int8)
    lab16 = lab8.astype(ml_dtypes.bfloat16)
    wcat = make_weights()
    p8 = [np.pad(lab8[b], ((2, 3), (2, 2))) for b in range(B)]
    p16 = [np.pad(lab16[b], ((2, 3), (2, 2))) for b in range(B)]
    rows128 = np.arange(128)
    in_maps = []
    for c in range(NCORES):
        b, q = divmod(c, 4)
        xf = p8[b]
        base = 248 * q
        x = np.concatenate([xf[base + rows128, :],
                            xf[base + 1 + rows128, :],
                            xf[base + 124 + rows128, :],
                            xf[base + 125 + rows128, :]], axis=1)
        cs = 256 * c
        st_cur = np.concatenate([p16[0][992:1028, cs:cs + STW],
                                 p16[1][992:1028, cs:cs + STW]], axis=0)
        st_nxt = np.concatenate([p16[0][993:1029, cs:cs + STW],
                                 p16[1][993:1029, cs:cs + STW]], axis=0)
        stc = np.concatenate([st_cur, st_nxt], axis=1)
        in_maps.append({"x8": np.ascontiguousarray(x),
                        "st": np.ascontiguousarray(stc), "wcat": wcat})
    return in_maps


def _unpack(Bts):
    # [nb, C] uint8 -> [8*nb, C] bits (little: bit j = row 8i+j)
    return (((Bts[:, None, :].astype(np.uint16) >> np.arange(8)[None, :, None])
             & 1).reshape(-1, Bts.shape[1]).astype(np.int32))


def assemble(results):
    out = np.zeros((B, 1, H, W), np.int32)
    for c in range(NCORES):
        b, q = divmod(c, 4)
        P = results[c]["y2p"]
        CHMAP = {0: (0, 0), 1: (1, 0), 2: (2, 0), 3: (0, 1),
                 4: (1, 1), 5: (2, 1), 6: (0, 2), 7: (1, 2)}
        for k in range(8):
            slot, reg = CHMAP[k]
            ch = _unpack(P[16 * slot:16 * slot + 16,
                           512 * reg:512 * reg + 512])[2:126]
            if k < 4:
                out[b, 0, 248 * q:248 * q + 124, 512 * k:512 * (k + 1)] = ch
            else:
                out[b, 0, 248 * q + 124:248 * q + 248,
                    512 * (k - 4):512 * (k - 3)] = ch
        sp = _unpack(P[32:41, 1024:1280])[0:72]                # [72, 256]
        out[0, 0, 992:1024, 256 * c:256 * c + 256] = sp[2:34]
        out[1, 0, 992:1024, 256 * c:256 * c + 256] = sp[38:70]
    return out


def kernel(gtmasks):
    global LAST_EXEC_NS, LAST_RESULTS
    in_maps = make_in_maps(gtmasks)
    nc = build_nc()
    res = bass_utils.run_bass_kernel_spmd(
        nc, in_maps, core_ids=list(range(NCORES)), trace=PROFILE)
    LAST_EXEC_NS = res.exec_time_ns
    LAST_RESULTS = res
    return assemble(res.results)


# revision 5
# speedup vs baseline: 1.0226x; 1.0226x over previous
"""Boundary-map kernel v3 for Trainium2 (Bass, raw engine streams), 8-core SPMD.

Math (same as baseline): edge = (sum of 16 tap-shifted DH/DV indicator terms)
> 0 over the 2-padded label image; vertical taps via band-matrix matmuls,
horizontal via DVE shifted adds. Output e1 holds the raw tap COUNTS (0..16);
the host thresholds with != 0.

Per-core layout: seg1/seg2 = two 128-partition row bands (124 output rows
each, 2+2 halo), strip = bottom 32 rows of both batches as [72 x 260].
Host supplies next-row copies (duplication) so DV is a free-dim not_equal.

Input path: labels are sent as INT8 and expanded to bf16 by Pool-SWDGE
casting DMAs, which fan out across all 16 DMA engines (~2x the per-ring
HWDGE dispatch rate) and halve HBM traffic chip-wide. HW rings carry only
the bf16 weights (SP) and strip tile (ACT). Thresholds are ACT Copy ops
(no activation-table load). POOL does no tensor ops (first-op Q7 library
load costs ~8.5us). 7 manual semaphores; both segs 5-pass (w11*H2(0),
wi*H2(-1), wi*H2(+1), wv4*DV(0), wv2*DVHp(-1)); strip 4-pass into pA bank0.
"""

import numpy as np
import ml_dtypes

import concourse.bass as bass
import concourse.bacc as bacc
import concourse.mybir as mybir
from concourse import bass_utils

BF16 = mybir.dt.bfloat16
F32 = mybir.dt.float32
I8 = mybir.dt.int8
U8 = mybir.dt.uint8
OP = mybir.AluOpType
AF = mybir.ActivationFunctionType

B, H, W = 2, 1024, 2048
NCORES = 8
CHUNK = 512
SEGW = 2052
STW = 260
T = 2 * SEGW           # strip base col in intermediates
IW = 2 * SEGW + STW

PROFILE = False
LAST_EXEC_NS = None
LAST_RESULTS = None

N_DUMMY = 22


def _band(taps, P=128):
    w = np.zeros((P, P), np.float32)
    for m in range(P):
        for t in taps:
            k = m + t
            if 0 <= k < P:
                w[k, m] += 1.0
    return w


def _sband(taps):
    w36 = _band(taps, P=36)
    w = np.zeros((72, 72), np.float32)
    w[0:36, 0:36] = w36
    w[36:72, 36:72] = w36
    return w


TAPS = {"w_11": [-1, 1], "w_v4": [-2, -1, 0, 1], "w_v2": [-1, 0], "w_i": [0]}
WNAMES = ("w_11", "w_v4", "w_v2", "w_i")


def make_weights():
    big = np.concatenate([_band(TAPS[n]) for n in WNAMES], axis=1)
    strip = np.zeros((128, 72 * len(WNAMES)), np.float32)
    for i, n in enumerate(WNAMES):
        strip[0:72, 72 * i:72 * (i + 1)] = _sband(TAPS[n])
    pack = np.zeros((128, 16), np.float32)
    for i in range(16):
        for j in range(8):
            pack[8 * i + j, i] = float(1 << j)
    return np.concatenate([big, strip, pack], axis=1).astype(ml_dtypes.bfloat16)


def build_nc():
    nc = bacc.Bacc("TRN2", target_bir_lowering=False, debug=False)
    x8 = nc.dram_tensor("x8", [128, 4 * SEGW], I8, kind="ExternalInput").ap()
    st = nc.dram_tensor("st", [72, 2 * STW], BF16, kind="ExternalInput").ap()
    wcat = nc.dram_tensor("wcat", [128, 512 + 72 * 4 + 16], BF16,
                          kind="ExternalInput").ap()
    y2p = nc.dram_tensor("y2p", [48, 1536], U8, kind="ExternalOutput").ap()

    xi = nc.alloc_sbuf_tensor("xi", [128, 4 * SEGW], BF16)
    ST = nc.alloc_sbuf_tensor("ST", [72, 2 * STW], BF16)
    wt = nc.alloc_sbuf_tensor("wt", [128, 512 + 72 * 4 + 16], BF16)
    DH = nc.alloc_sbuf_tensor("DH", [128, IW], BF16)
    H2 = nc.alloc_sbuf_tensor("H2", [128, IW], BF16)
    DV = nc.alloc_sbuf_tensor("DV", [128, IW], BF16)
    DVHp = nc.alloc_sbuf_tensor("DVHp", [128, IW], BF16)
    H4p = nc.alloc_sbuf_tensor("H4p", [128, IW], BF16)
    e1 = nc.alloc_sbuf_tensor("e1", [128, 4352], BF16)
    e1p = nc.alloc_sbuf_tensor("e1p", [128, 1536], U8)
    pA = nc.alloc_psum_tensor("pA", [128, 2048], F32)
    pB = nc.alloc_psum_tensor("pB", [128, 2048], F32)

    wb = {n: wt[:, 128 * i:128 * (i + 1)] for i, n in enumerate(WNAMES)}
    ws = {n: wt[0:72, 512 + 72 * i:512 + 72 * (i + 1)]
          for i, n in enumerate(WNAMES)}
    wpk = wt[:, 800:816]

    s1 = nc.alloc_semaphore("s1")        # SP ring (wcat)
    s2 = nc.alloc_semaphore("s2")        # ACT ring (strip)
    s3 = nc.alloc_semaphore("s3")        # SWDGE casting DMAs
    vsem = nc.alloc_semaphore("vsem")
    psem = nc.alloc_semaphore("psem")
    asem = nc.alloc_semaphore("asem")
    osem = nc.alloc_semaphore("osem")

    with nc.Block(no_gpsimd_drain=True) as blk:

        @blk.sync
        def _(eng):
            eng.dma_start(wt[:, :], wcat).then_inc(s1, 16)
            eng.wait_ge(asem, 6)
            eng.wait_ge(vsem, 18)
            eng.dma_start(y2p[16:32, :], e1p[32:48, :]).then_inc(osem, 16)
            eng.wait_ge(osem, 48)

        @blk.gpsimd
        def _(eng):
            # SWDGE casting DMAs: int8 HBM -> bf16 SBUF, fan out over all
            # 16 DMA engines; cur halves first (they gate the DH/H2 path)
            eng.dma_start(xi[:, 0:SEGW], x8[:, 0:SEGW]).then_inc(s3, 16)
            eng.dma_start(xi[:, 2 * SEGW:3 * SEGW],
                          x8[:, 2 * SEGW:3 * SEGW]).then_inc(s3, 16)
            eng.dma_start(xi[:, SEGW:2 * SEGW],
                          x8[:, SEGW:2 * SEGW]).then_inc(s3, 16)
            eng.dma_start(xi[:, 3 * SEGW:4 * SEGW],
                          x8[:, 3 * SEGW:4 * SEGW]).then_inc(s3, 16)
            # bit-packed output, partition-group slice on each queue
            eng.wait_ge(asem, 6)
            eng.wait_ge(vsem, 18)
            eng.dma_start(y2p[0:16, :], e1p[0:16, :]).then_inc(osem, 16)

        @blk.scalar
        def _(eng):
            eng.dma_start(ST[:, :], st[:, :]).then_inc(s2, 16)
            # threshold copies split with DVE: ACT takes strip (retires pB
            # bank 3 for seg2 c3) + c0/c1 of each seg; DVE takes c2/c3
            eng.wait_ge(psem, 1)
            eng.activation(out=e1[0:72, 4096:4352], in_=pB[0:72, 1536:1792],
                           func=AF.Sign).then_inc(asem, 1)
            for k in range(2):
                eng.wait_ge(psem, 2 + k)
                eng.activation(out=e1[:, k * CHUNK:(k + 1) * CHUNK],
                               in_=pA[:, k * CHUNK:(k + 1) * CHUNK],
                               func=AF.Sign).then_inc(asem, 1)
            for k in range(2):
                eng.wait_ge(psem, 6 + k)
                eng.activation(out=e1[:, 2048 + k * CHUNK:2048 + (k + 1) * CHUNK],
                               in_=pB[:, k * CHUNK:(k + 1) * CHUNK],
                               func=AF.Sign).then_inc(asem, 1)
            # packed result -> uint8 (garbage in the partition gaps; the
            # host reads only the three 16-partition slices)
            eng.wait_ge(psem, 10)
            eng.copy(out=e1p[:, 0:1024], in_=pA[:, 0:1024]).then_inc(asem, 1)
            eng.wait_ge(vsem, 18)
            eng.dma_start(y2p[32:48, :], e1p[64:80, :]).then_inc(osem, 16)

        @blk.vector
        def _(eng):
            # strip chain first: its tiny input lands well before the big
            # casted segments, so it hides entirely in the input window
            eng.wait_ge(s2, 16)
            eng.tensor_tensor(out=DH[0:72, T:T + 259], in0=ST[:, 0:259],
                              in1=ST[:, 1:260],
                              op=OP.not_equal).then_inc(vsem, 1)   # v1 SDH
            eng.tensor_tensor(out=H2[0:72, T + 1:T + 259],
                              in0=DH[0:72, T:T + 258],
                              in1=DH[0:72, T + 1:T + 259],
                              op=OP.add).then_inc(vsem, 1)         # v2 SH2
            eng.tensor_tensor(out=DV[0:72, T:T + 260], in0=ST[:, 0:260],
                              in1=ST[:, 260:520],
                              op=OP.not_equal).then_inc(vsem, 1)   # v3 SDV
            eng.tensor_tensor(out=DVHp[0:72, T:T + 258],
                              in0=DV[0:72, T:T + 258],
                              in1=DV[0:72, T + 2:T + 260],
                              op=OP.add).then_inc(vsem, 1)         # v4 SDVHp
            eng.tensor_tensor(out=H4p[0:72, T + 1:T + 257],
                              in0=H2[0:72, T + 1:T + 257],
                              in1=H2[0:72, T + 3:T + 259],
                              op=OP.add).then_inc(vsem, 1)         # v5 SH4p
            # H2 paths for BOTH segs first (cur halves arrive first), then
            # the DV paths -- feeds PE's interleaved group order
            eng.wait_ge(s3, 16)
            eng.tensor_tensor(out=DH[:, 0:2051], in0=xi[:, 0:2051],
                              in1=xi[:, 1:2052],
                              op=OP.not_equal).then_inc(vsem, 1)   # v6 DH1
            eng.tensor_tensor(out=H2[:, 1:2051],
                              in0=DH[:, 0:2050],
                              in1=DH[:, 1:2051],
                              op=OP.add).then_inc(vsem, 1)         # v7 H2-1
            S = SEGW
            eng.wait_ge(s3, 32)
            eng.tensor_tensor(out=DH[:, S:S + 2051],
                              in0=xi[:, 2 * SEGW:2 * SEGW + 2051],
                              in1=xi[:, 2 * SEGW + 1:2 * SEGW + 2052],
                              op=OP.not_equal).then_inc(vsem, 1)   # v8 DH2
            eng.tensor_tensor(out=H2[:, S + 1:S + 2051],
                              in0=DH[:, S:S + 2050],
                              in1=DH[:, S + 1:S + 2051],
                              op=OP.add).then_inc(vsem, 1)         # v9 H2-2
            eng.wait_ge(s3, 48)
            eng.tensor_tensor(out=DV[:, 0:2052], in0=xi[:, 0:2052],
                              in1=xi[:, SEGW:SEGW + 2052],
                              op=OP.not_equal).then_inc(vsem, 1)   # v10 DV1
            eng.tensor_tensor(out=DVHp[:, 0:2050],
                              in0=DV[:, 0:2050],
                              in1=DV[:, 2:2052],
                              op=OP.add).then_inc(vsem, 1)         # v11 DVHp1
            eng.wait_ge(s3, 64)
            eng.tensor_tensor(out=DV[:, S:S + 2052],
                              in0=xi[:, 2 * SEGW:2 * SEGW + 2052],
                              in1=xi[:, 3 * SEGW:3 * SEGW + 2052],
                              op=OP.not_equal).then_inc(vsem, 1)   # v12 DV2
            eng.tensor_tensor(out=DVHp[:, S:S + 2050],
                              in0=DV[:, S:S + 2050],
                              in1=DV[:, S + 2:S + 2052],
                              op=OP.add).then_inc(vsem, 1)         # v13 DVHp2
            # c2/c3 threshold copies of both segs on the now-idle DVE, in
            # parallel with ACT's strip/c0/c1 copies
            for S, ps, pw in ((1024, pA, 4), (1536, pA, 5),
                              (3072, pB, 8), (3584, pB, 9)):
                eng.wait_ge(psem, pw)
                eng.tensor_scalar(out=e1[:, S:S + CHUNK],
                                  in0=ps[:, S % 2048:S % 2048 + CHUNK],
                                  scalar1=0.0, scalar2=None,
                                  op0=OP.is_gt).then_inc(vsem, 1)  # v14-17
            eng.wait_ge(psem, 10)
            eng.tensor_scalar(out=e1p[:, 1024:1536], in0=pA[:, 1024:1536],
                              scalar1=0.0, scalar2=None,
                              op0=OP.bypass).then_inc(vsem, 1)     # v18

        @blk.tensor
        def _(eng):
            for i in range(N_DUMMY):
                eng.matmul(out=pB[:, 1536:2048], lhsT=H4p[0:128, 512:640],
                           rhs=H4p[0:128, 1024:1536], start=True, stop=True,
                           skip_group_check=True)
            # strip: 4 passes into pB bank 3, right after the dummies; ACT
            # copies it out (asem 1) long before seg2-c3 reuses the bank
            eng.wait_ge(s1, 16)
            eng.wait_ge(vsem, 5)
            a = T + 2
            for wi, (wn, rhs, doff) in enumerate(
                    [("w_11", H2, 0), ("w_v4", DV, 0),
                     ("w_v2", DVHp, -1), ("w_i", H4p, -1)]):
                mm = eng.matmul(out=pB[0:72, 1536:1792], lhsT=ws[wn],
                                rhs=rhs[0:72, a + doff:a + doff + 256],
                                start=(wi == 0), stop=(wi == 3),
                                skip_group_check=True)
            mm.then_inc(psem, 1)

            def seg_group(seg, wn, rhs, doff, stop=False, p0=None):
                S, ps = (0, pA) if seg == 1 else (SEGW, pB)
                for k in range(4):
                    if seg == 2 and wi_guard[0] and k == 3:
                        eng.wait_ge(asem, 1)
                        wi_guard[0] = False
                    a = S + 2 + doff + k * CHUNK
                    mm = eng.matmul(out=ps[:, k * CHUNK:(k + 1) * CHUNK],
                                    lhsT=wb[wn],
                                    rhs=rhs[0:128, a:a + CHUNK],
                                    start=(p0 is not None and p0 == "start"),
                                    stop=stop, skip_group_check=True)
                    if stop:
                        mm.then_inc(psem, 1)

            wi_guard = [True]
            # interleaved: H2-dependent groups of both segs first, DV/DVHp
            # groups as DVE produces them
            eng.wait_ge(vsem, 7)
            seg_group(1, "w_11", H2, 0, p0="start")
            seg_group(1, "w_i", H2, -1)
            seg_group(1, "w_i", H2, 1)
            eng.wait_ge(vsem, 9)
            seg_group(2, "w_11", H2, 0, p0="start")
            seg_group(2, "w_i", H2, -1)
            seg_group(2, "w_i", H2, 1)
            eng.wait_ge(vsem, 10)
            seg_group(1, "w_v4", DV, 0)
            eng.wait_ge(vsem, 11)
            seg_group(1, "w_v2", DVHp, -1, stop=True)
            eng.wait_ge(vsem, 12)
            seg_group(2, "w_v4", DV, 0)
            eng.wait_ge(vsem, 13)
            seg_group(2, "w_v2", DVHp, -1, stop=True)
            # bit-pack passes: out rows 8i..8i+7 of each 512-col chunk fold
            # into byte-partition 16k+i of pA banks 0/1 (free after their
            # threshold reads)
            # matmul out base partitions limited to {0, 32, 64}: chunk ->
            # (slot, region) of pA banks 0-2, strip at (2, 2); banks are
            # reused only after their threshold copy retired them
            packs = [(0, 0, 0, "a", 2), (1, 1, 0, "a", 3), (2, 2, 0, "v", 14),
                     (8, 2, 2, None, 0), (3, 0, 1, "v", 15),
                     (4, 1, 1, "a", 4), (5, 2, 1, "a", 5),
                     (6, 0, 2, "v", 16), (7, 1, 2, "v", 17)]
            for k, slot, reg, sem, lvl in packs:
                if sem == "a":
                    eng.wait_ge(asem, lvl)
                elif sem == "v":
                    eng.wait_ge(vsem, lvl)
                base, off = 32 * slot, 512 * reg
                if k == 8:  # strip
                    mm = eng.matmul(out=pA[base:base + 9, off:off + 256],
                                    lhsT=wpk[0:72, 0:9],
                                    rhs=e1[0:72, 4096:4352], start=True,
                                    stop=True, skip_group_check=True)
                else:
                    mm = eng.matmul(out=pA[base:base + 16, off:off + 512],
                                    lhsT=wpk[0:128, 0:16],
                                    rhs=e1[0:128, 512 * k:512 * (k + 1)],
                                    start=True, stop=True,
                                    skip_group_check=True)
            mm.then_inc(psem, 1)

    nc.compile()
    return nc


def make_in_maps(gtmasks):
    lab8 = np.asarray(gtmasks)[:, 0].astype(np.int8)
    lab16 = lab8.astype(ml_dtypes.bfloat16)
    wcat = make_weights()
    p8 = [np.pad(lab8[b], ((2, 3), (2, 2))) for b in range(B)]
    p16 = [np.pad(lab16[b], ((2, 3), (2, 2))) for b in range(B)]
    rows128 = np.arange(128)
    in_maps = []
    for c in range(NCORES):
        b, q = divmod(c, 4)
        xf = p8[b]
        base = 248 * q
        x = np.concatenate([xf[base + rows128, :],
                            xf[base + 1 + rows128, :],
                            xf[base + 124 + rows128, :],
                            xf[base + 125 + rows128, :]], axis=1)
        cs = 256 * c
        st_cur = np.concatenate([p16[0][992:1028, cs:cs + STW],
                                 p16[1][992:1028, cs:cs + STW]], axis=0)
        st_nxt = np.concatenate([p16[0][993:1029, cs:cs + STW],
                                 p16[1][993:1029, cs:cs + STW]], axis=0)
        stc = np.concatenate([st_cur, st_nxt], axis=1)
        in_maps.append({"x8": np.ascontiguousarray(x),
                        "st": np.ascontiguousarray(stc), "wcat": wcat})
    return in_maps


def _unpack(Bts):
    # [nb, C] uint8 -> [8*nb, C] bits (little: bit j = row 8i+j)
    return (((Bts[:, None, :].astype(np.uint16) >> np.arange(8)[None, :, None])
             & 1).reshape(-1, Bts.shape[1]).astype(np.int32))


def assemble(results):
    out = np.zeros((B, 1, H, W), np.int32)
    for c in range(NCORES):
        b, q = divmod(c, 4)
        P = results[c]["y2p"]
        CHMAP = {0: (0, 0), 1: (1, 0), 2: (2, 0), 3: (0, 1),
                 4: (1, 1), 5: (2, 1), 6: (0, 2), 7: (1, 2)}
        for k in range(8):
            slot, reg = CHMAP[k]
            ch = _unpack(P[16 * slot:16 * slot + 16,
                           512 * reg:512 * reg + 512])[2:126]
            if k < 4:
                out[b, 0, 248 * q:248 * q + 124, 512 * k:512 * (k + 1)] = ch
            else:
                out[b, 0, 248 * q + 124:248 * q + 248,
                    512 * (k - 4):512 * (k - 3)] = ch
        sp = _unpack(P[32:41, 1024:1280])[0:72]                # [72, 256]
        out[0, 0, 992:1024, 256 * c:256 * c + 256] = sp[2:34]
        out[1, 0, 992:1024, 256 * c:256 * c + 256] = sp[38:70]
    return out


def kernel(gtmasks):
    global LAST_EXEC_NS, LAST_RESULTS
    in_maps = make_in_maps(gtmasks)
    nc = build_nc()
    res = bass_utils.run_bass_kernel_spmd(
        nc, in_maps, core_ids=list(range(NCORES)), trace=PROFILE)
    LAST_EXEC_NS = res.exec_time_ns
    LAST_RESULTS = res
    return assemble(res.results)
